# revision 1
# baseline (speedup 1.0000x reference)
"""Multi-head attention (B=2, N=2048, C=1024, H=16, D=64) on 8 TRN2 NeuronCores.

Sharding: data-parallel over batch (cores 0-3 -> b=0, cores 4-7 -> b=1),
tensor-parallel over heads (4 heads per core: columns of Wq/Wkv, rows of Wo).
Each core computes a partial output projection y_partial[b] summed over its
4 heads; the host reduces the 4 partials per batch and adds the bias bo.

Device layout notes (per core):
  - All matmuls run as float32r (TF32-like single-pass fp32 mode, ~4x fp32).
  - x is passed transposed (xT [C, N]); q,k are produced transposed
    (qT/kT [256, N], head h at partition offset (h%2)*64, chunk h//2);
    v is produced in natural layout [N, 256] with a ones column appended per
    head so the attention row-sum rides along the attn@v matmul.
  - Scores are computed transposed, ST[keys, queries] = kT_h.T-free matmul,
    attn_bias arrives pre-transposed from the host (biasT[h, m, n]) and is
    added into PSUM with an identity matmul; exp runs on the ACT engine
    straight out of PSUM (softmax max-subtraction is skipped: |scores| < ~20
    so exp cannot overflow and softmax is shift-invariant).
  - attn@v: UT'[65, n] accumulates over key chunks; row 64 is the softmax
    denominator. Normalization multiplies by the broadcast reciprocal.
  - Output projection contracts per-head (K=64) so every tile sits at
    partition base 0.
"""

import os

import numpy as np

import concourse.bass as bass
import concourse.tile as tile
from concourse import bacc, mybir
from concourse.bass_utils import run_bass_kernel_spmd
from concourse.masks import make_identity

B, N, C = 2, 2048, 1024
H, D = 16, 64
HLOC = 4          # heads per core
HD = HLOC * D     # 256 channels per core
SCALE = D ** -0.5
P = 128
KCH = C // P      # 8 k-chunks for the projections
NT = N // P       # 16 token / key chunks of 128
NQ = N // 512     # 4 query chunks of 512
F32 = mybir.dt.float32
F16 = mybir.dt.float16
MM_DT = mybir.dt.float32r

_NC_CACHE = {}


def build_nc(reps=1):
    nc = bacc.Bacc("TRN2", target_bir_lowering=False, debug=False)

    xT = nc.dram_tensor("xT", [C, N], MM_DT, kind="ExternalInput")
    wqT = nc.dram_tensor("wqT", [C, HD], MM_DT, kind="ExternalInput")
    wkT = nc.dram_tensor("wkT", [C, HD], MM_DT, kind="ExternalInput")
    wvT = nc.dram_tensor("wvT", [C, HD], MM_DT, kind="ExternalInput")
    woT = nc.dram_tensor("woT", [HD, C], MM_DT, kind="ExternalInput")
    biasT = nc.dram_tensor("biasT", [HLOC, N, N], F16, kind="ExternalInput")
    y = nc.dram_tensor("y", [N, C], F32, kind="ExternalOutput")

    with tile.TileContext(nc) as tc:
        with (
            tc.tile_pool(name="consts", bufs=1) as consts,
            tc.tile_pool(name="wpool", bufs=1) as wpool,
            tc.tile_pool(name="xt", bufs=2) as xtp,
            tc.tile_pool(name="qk", bufs=1) as qkp,
            tc.tile_pool(name="bias", bufs=4) as biasp,
            tc.tile_pool(name="et", bufs=2) as etp,
            tc.tile_pool(name="norm", bufs=1) as normp,
            tc.tile_pool(name="ysb", bufs=1) as ysbp,
            tc.tile_pool(name="ps", bufs=2, space="PSUM") as psp,
            tc.tile_pool(name="pu", bufs=4, space="PSUM") as pup,
        ):
            # ---- constants / weights ----
            ones16 = consts.tile([P, NT], F32)
            nc.vector.memset(ones16, 1.0)

            wq_sb = wpool.tile([P, KCH, HD], MM_DT)
            nc.sync.dma_start(wq_sb, wqT[:, :].rearrange("(ko p) m -> p ko m", p=P))
            wk_sb = wpool.tile([P, KCH, HD], MM_DT)
            nc.sync.dma_start(wk_sb, wkT[:, :].rearrange("(ko p) m -> p ko m", p=P))
            wv_sb = wpool.tile([P, KCH, HD], MM_DT)
            nc.sync.dma_start(wv_sb, wvT[:, :].rearrange("(ko p) m -> p ko m", p=P))
            wo_sb = wpool.tile([D, HLOC, C], MM_DT)
            nc.sync.dma_start(wo_sb, woT[:, :].rearrange("(h p) n -> p h n", p=D))

            qT_sb = qkp.tile([P, 2, N], MM_DT)
            kT_sb = qkp.tile([P, 2, N], MM_DT)
            v_sb = qkp.tile([P, NT, HLOC, D + 1], MM_DT)
            outT_sb = qkp.tile([D, HLOC, N], MM_DT)

            # ones column of v (softmax denominator rides the attn@v matmul)
            for h in range(HLOC):
                nc.vector.tensor_copy(v_sb[:, :, h, D], ones16)

            for _rep in range(reps):
                # ---- stage 1: projections (qT, kT transposed; v natural) ----
                xT_r = xT[:, :].rearrange("(ko p) n -> p ko n", p=P)
                TC = 256  # token chunk
                for t in range(N // TC):
                    xt = xtp.tile([P, KCH, TC], MM_DT)
                    nc.sync.dma_start(xt, xT_r[:, :, t * TC:(t + 1) * TC])
                    for mo in range(2):
                        pq = psp.tile([P, 1024], F32, tag="ps", name="pq")[:, :TC]
                        pk = psp.tile([P, 1024], F32, tag="ps", name="pk")[:, :TC]
                        for k in range(KCH):
                            nc.tensor.matmul(
                                pq, lhsT=wq_sb[:, k, mo * P:(mo + 1) * P],
                                rhs=xt[:, k, :], start=(k == 0), stop=(k == KCH - 1))
                        for k in range(KCH):
                            nc.tensor.matmul(
                                pk, lhsT=wk_sb[:, k, mo * P:(mo + 1) * P],
                                rhs=xt[:, k, :], start=(k == 0), stop=(k == KCH - 1))
                        nc.vector.tensor_copy(qT_sb[:, mo, t * TC:(t + 1) * TC], pq)
                        nc.vector.tensor_copy(kT_sb[:, mo, t * TC:(t + 1) * TC], pk)
                    for sub in range(TC // P):
                        mt = (t * TC) // P + sub
                        pv = psp.tile([P, 1024], F32, tag="ps", name="pv")[:, :HD]
                        for k in range(KCH):
                            nc.tensor.matmul(
                                pv, lhsT=xt[:, k, sub * P:(sub + 1) * P],
                                rhs=wv_sb[:, k, :], start=(k == 0), stop=(k == KCH - 1))
                        nc.vector.tensor_copy(
                            v_sb[:, mt, :, 0:D],
                            pv.rearrange("p (h d) -> p h d", h=HLOC))

                # ---- stage 2: attention per head ----
                for h in range(HLOC):
                    p0 = (h % 2) * D
                    ch = h // 2
                    pu_t = [pup.tile([D + 1, 512], F32, tag="pu", name=f"pu{h}_{i}") for i in range(NQ)]
                    for mk in range(NT):
                        bt = biasp.tile([P, N], F16, tag="bias")
                        nc.sync.dma_start(bt, biasT[h, mk * P:(mk + 1) * P, :])
                        et = etp.tile([P, N], MM_DT, tag="et")
                        for j in range(2):
                            ps_s = psp.tile([P, 1024], F32, tag="ps")
                            for q2 in range(2):
                                nqi = j * 2 + q2
                                sl = slice(q2 * 512, (q2 + 1) * 512)
                                gl = slice(nqi * 512, (nqi + 1) * 512)
                                nc.tensor.matmul(
                                    ps_s[:, sl],
                                    lhsT=kT_sb[p0:p0 + D, ch, mk * P:(mk + 1) * P],
                                    rhs=qT_sb[p0:p0 + D, ch, gl],
                                    start=True, stop=True)
                            nc.vector.tensor_add(
                                ps_s, ps_s, bt[:, j * 1024:(j + 1) * 1024])
                            nc.scalar.activation(
                                et[:, j * 1024:(j + 1) * 1024], ps_s,
                                mybir.ActivationFunctionType.Exp)
                        for nqi in range(NQ):
                            nc.tensor.matmul(
                                pu_t[nqi], lhsT=v_sb[:, mk, h, :],
                                rhs=et[:, nqi * 512:(nqi + 1) * 512],
                                start=(mk == 0), stop=(mk == NT - 1))
                    # normalize: outT_h = UT / r
                    r_recip = normp.tile([P, N], F32, tag="rr")
                    for nqi in range(NQ):
                        nc.vector.reciprocal(
                            r_recip[D:D + 1, nqi * 512:(nqi + 1) * 512],
                            pu_t[nqi][D:D + 1, :])
                    # partition_broadcast reads the tensor's literal partition 0 on
                    # HW (AP base-partition offsets are ignored), so stage r there.
                    r0 = normp.tile([1, N], F32, tag="r0")
                    nc.vector.tensor_copy(r0[0:1, :], r_recip[D:D + 1, :])
                    bcast = normp.tile([D, N], F32, tag="bc")
                    nc.gpsimd.partition_broadcast(bcast, r0[0:1, :])
                    for nqi in range(NQ):
                        nc.vector.tensor_mul(
                            outT_sb[:, h, nqi * 512:(nqi + 1) * 512],
                            pu_t[nqi][0:D, :],
                            bcast[:, nqi * 512:(nqi + 1) * 512])

                # ---- stage 3: output projection (partial y, summed over 4 heads) ----
                for mt in range(NT):
                    py = psp.tile([P, 1024], F32, tag="ps")
                    for j in range(2):
                        for h in range(HLOC):
                            nc.tensor.matmul(
                                py[:, j * 512:(j + 1) * 512],
                                lhsT=outT_sb[:, h, mt * P:(mt + 1) * P],
                                rhs=wo_sb[:, h, j * 512:(j + 1) * 512],
                                start=(h == 0), stop=(h == HLOC - 1))
                    y_t = ysbp.tile([P, 1024], F32, tag="y")
                    nc.vector.tensor_copy(y_t, py)
                    nc.sync.dma_start(y[mt * P:(mt + 1) * P, :], y_t)

    nc.compile()
    return nc


def _get_nc():
    if "nc" not in _NC_CACHE:
        _NC_CACHE["nc"] = build_nc()
    return _NC_CACHE["nc"]


def _shard_inputs(x, attn_bias, Wq, Wkv, Wo):
    in_maps = []
    for core in range(8):
        b = core // 4
        hg = core % 4
        rows = slice(hg * HD, (hg + 1) * HD)
        in_maps.append({
            "xT": np.ascontiguousarray(x[b].T),
            "wqT": np.ascontiguousarray((Wq[rows, :] * SCALE).T),
            "wkT": np.ascontiguousarray(Wkv[rows, :].T),
            "wvT": np.ascontiguousarray(Wkv[C + rows.start:C + rows.stop, :].T),
            "woT": np.ascontiguousarray(Wo[:, rows].T),
            "biasT": np.ascontiguousarray(
                attn_bias[b, hg * HLOC:(hg + 1) * HLOC].transpose(0, 2, 1)
            ).astype(np.float16),
        })
    return in_maps


def run(inputs, trace=False):
    x = np.asarray(inputs["x"], dtype=np.float32)
    attn_bias = np.asarray(inputs["attn_bias"], dtype=np.float32)
    Wq = np.asarray(inputs["Wq"], dtype=np.float32)
    Wkv = np.asarray(inputs["Wkv"], dtype=np.float32)
    Wo = np.asarray(inputs["Wo"], dtype=np.float32)
    bo = np.asarray(inputs["bo"], dtype=np.float32)

    nc = _get_nc()
    in_maps = _shard_inputs(x, attn_bias, Wq, Wkv, Wo)
    if trace:
        res = run_bass_kernel_spmd(nc, in_maps, core_ids=list(range(8)), trace=True)
    else:
        # The axon NTFF profiling hook is unavailable in this container; make
        # sure a stray BASS_TRACE env can't send us down that path.
        prev = os.environ.get("BASS_NEVER_TRACE")
        os.environ["BASS_NEVER_TRACE"] = "1"
        try:
            res = run_bass_kernel_spmd(nc, in_maps, core_ids=list(range(8)),
                                       trace=False)
        finally:
            if prev is None:
                os.environ.pop("BASS_NEVER_TRACE", None)
            else:
                os.environ["BASS_NEVER_TRACE"] = prev

    y = np.zeros((B, N, C), dtype=np.float32)
    for core in range(8):
        y[core // 4] += res.results[core]["y"]
    y += bo[None, None, :]
    return y, res.exec_time_ns


def kernel(**inputs):
    out, _ = run(inputs, trace=False)
    return out

